# revision 34
# baseline (speedup 1.0000x reference)
"""BasisResidualFFN Trainium2 kernel.

Math (per token t):
  recipe_soft = softmax(neuron_recipe, axis=-1)                 [64, 16]
  tr[t, :]    = sum_k w[t,k] * recipe_soft[idx[t,k], :]         [16]
  Y[t, (n,r)] = sum_d x[t,d] * basis_A[n,d,r]
  h[t, r]     = sum_n tr[t,n] * Y[t,(n,r)]
  delta[t, d] = sum_{n,r} basis_A[n,d,r] * tr[t,n] * h[t,r]
  out         = gelu((x + alpha*delta) @ w_up + b_up) @ w_down + b_down

Distribution: pure data parallel. B*S = 4096 tokens sharded 512/core
across 8 NeuronCores; all weights replicated. Everything on device is
computed feature-major (features on partitions, tokens on the free
axis, 512 tokens per matmul) so no on-device activation transposes are
needed anywhere in the FFN; x arrives pre-transposed from the host and
the output is un-transposed on the host.

Precision: the FFN runs bf16 (it dominates the error budget). The
basis/routing path runs fp8e4 with DoubleRow matmuls (2x PE throughput
when the HAM clock is at 8/8) -- its errors enter the output only
through alpha*delta with alpha ~ 0.1, so they are strongly damped
(measured 0.40% rel err end to end vs 0.345% for all-bf16).

fp8 scale chain (all powers of two, folded into host constants):
  x8 = 16*x, a1 = 256*A1  ->  YT_psum = 4096*Y
  SEL *= 2^-12            ->  repr = tr/4096, wyt = Y*tr (bf16)
  M = 32768*[p'%32==p%32] ->  rh_psum = sum_i M^T wyt_i = 32768*h
                              (ht and its copy are folded into one
                               accumulated matmul)
  ct = rh (.) repr = 8*tr*h  (fp8, |ct| < 26 << 240 = trn fp8e4 max)
  a2 = 1024*alpha*A2      ->  dl_psum = 8192*alpha*delta
  xtb = 8192*x (bf16)     ->  xf = Copy(dl) + xtb = 8192*(x+a*delta)
  gelu(u) evaluated as Act(scale=2^-13) on u_psum = 8192*u.

Scheduling: PE is warmed with matmuls on a memset tile (no DMA
dependency) from ~8us, and small filler matmuls keep the HAM clock
gate at 8/8 through the DVE-bound stretches of the basis phase (an
idle PE is re-throttled to half clock, which would double the cost of
everything after). PSUM->SBUF copies are split between the Act engine
and the DVE so neither serializes the chain. The two HWDGE DMA rings
are loaded in strict need-order (each dma_start costs ~2us fixed +
bytes/rate, FIFO per ring), with all small constants merged into one
per-core blob.
"""

import numpy as np

import concourse.bass as bass
import concourse.mybir as mybir
import concourse.tile as tile
from concourse import bacc
from concourse.bass import ts
from concourse.bass_utils import run_bass_kernel_spmd

P = 128
NCORES = 8
T = 512            # tokens per core
D = 1024
DFF = 4096
NB = 16            # n_basis
R = 32             # rank
NN = 64            # n_neurons
K = 8              # top-k
DC = D // P        # 8 contraction chunks over d
FT = DFF // P      # 32 ff tiles
DT = D // P        # 8 output d tiles
NRT = (NB * R) // P  # 4 (n,r) tiles
TT = T // P        # 4 token tiles per core

# fp8 scale chain (powers of two)
SX = 16.0          # x fp8 scale
SA = 256.0         # a1 fp8 scale
S2 = 1024.0        # alpha*a2 fp8 scale
SIGR = 2.0 ** -12  # SEL scale  (= 1/(SX*SA))
SIGM = 32768.0     # M scale    (SIGM*SIGR = 8 = ct scale)
XS = 8192.0        # xtb prescale (= ct_scale * S2)
GS = 2.0 ** -13    # gelu input scale (= 1/XS)
F8MAX = 240.0      # trn2 fp8e4 max normal

# merged bf16 blob column layout (per-core: contains idx/weights);
# iota / identity / SEL tables are generated on-device
BB_IDX = 0
BB_QM = BB_IDX + 2 * K * TT        # 64
BB_REC = BB_QM + P                 # 192
BB_W = BB_REC + NB                 # 208
BF_BU, BF_BD, BF_W = 0, 32, 40     # f32 biases

F32 = mybir.dt.float32
BF16 = mybir.dt.bfloat16
FP8 = mybir.dt.float8e4

NWARM = 15

_BUILT = [None]


def _build_nc():
    nc = bacc.Bacc(None, target_bir_lowering=False)

    x8_d = nc.dram_tensor("x8", [P, DC, T], FP8, kind="ExternalInput")
    xtb_d = nc.dram_tensor("xtb", [P, DC, T], BF16, kind="ExternalInput")
    blobb_d = nc.dram_tensor("blobb", [P, BB_W], BF16, kind="ExternalInput")
    blobf_d = nc.dram_tensor("blobf", [P, BF_W], F32, kind="ExternalInput")
    a1_d = nc.dram_tensor("a1", [P, DC, NB * R], FP8, kind="ExternalInput")
    a2_d = nc.dram_tensor("a2", [P, NRT, D], FP8, kind="ExternalInput")
    wu_d = nc.dram_tensor("wu", [FT // 2, P, 2, DC, P], BF16, kind="ExternalInput")
    wd_d = nc.dram_tensor("wd", [DT * 2, P, FT // 2, P], BF16, kind="ExternalInput")
    out_d = nc.dram_tensor("outT", [P, DT, T], F32, kind="ExternalOutput")

    AX = mybir.AxisListType.X
    AF = mybir.ActivationFunctionType
    ALU = mybir.AluOpType
    DR = mybir.MatmulPerfMode.DoubleRow

    with tile.TileContext(nc) as tc:
        with (
            tc.tile_pool(name="const", bufs=1) as constp,
            tc.tile_pool(name="stream", bufs=4) as stream,
            tc.tile_pool(name="otp", bufs=3) as otp,
            tc.tile_pool(name="wdstream", bufs=4) as wdstream,
            tc.tile_pool(name="mid", bufs=1) as mid,
            tc.tile_pool(name="small", bufs=2) as small,
            tc.tile_pool(name="tmpp", bufs=3) as tmpp,
            tc.tile_pool(name="psum", bufs=4, space="PSUM") as psum,
            tc.tile_pool(name="psums", bufs=1, space="PSUM") as psums,
        ):
            # ---- PE warm-up on a memset tile: no DMA dependency, so the
            # HAM clock gate ramps from ~8us while the input DMAs land ----
            wz = constp.tile([P, T + P], BF16, tag="wz")
            nc.gpsimd.memset(wz[:], 0.0)
            # x8 rides the otherwise-idle SWDGE ring so neither HWDGE ring
            # carries two critical transfers
            x8 = constp.tile([P, DC, T], FP8, tag="x8")
            nc.gpsimd.dma_start(x8[:], x8_d[:])
            warm_ps = psums.tile([P, T], F32, tag="htps", name="warm")
            for w in range(NWARM):
                nc.tensor.matmul(warm_ps[:], wz[:, :P], wz[:, :T],
                                 start=(w == 0), stop=(w == NWARM - 1))

            def filler(name, n):
                # small matmuls that keep the HAM clock gate at 8/8 while
                # the PE waits on DVE stages; drained in ~110ns each once
                # real work unblocks.  Anchored via the idle Act engine.
                f_ps = psums.tile([P, T], F32, tag="htps", name=f"fill_{name}")
                for i in range(n):
                    nc.tensor.matmul(f_ps[:, :P], wz[:, :P], wz[:, T:T + P],
                                     start=(i == 0), stop=(i == n - 1))
                fa = tmpp.tile([P, 1], F32, tag="fanchor", name=f"fa_{name}")
                nc.scalar.activation(fa[:], f_ps[:, 0:1], AF.Copy)

            # on-device tables (GpSimd iota + DVE compare): iota512, the
            # transpose identity, and the SEL one-hot replication pattern.
            # Only iot gates the scatter; the rest are emitted after the
            # gpsimd scatter half so they don't delay it.
            iot = constp.tile([P, NN * K], BF16, tag="iot")
            nc.gpsimd.iota(iot[:], pattern=[[1, NN], [0, K]], base=0,
                           channel_multiplier=0,
                           allow_small_or_imprecise_dtypes=True)

            # remaining on-device tables
            colp = constp.tile([P, P], BF16, tag="colp")
            nc.gpsimd.iota(colp[:], pattern=[[1, P]], base=0,
                           channel_multiplier=0,
                           allow_small_or_imprecise_dtypes=True)
            pvec = constp.tile([P, 1], F32, tag="pvec")
            nc.gpsimd.iota(pvec[:], pattern=[[0, 1]], base=0,
                           channel_multiplier=1,
                           allow_small_or_imprecise_dtypes=True)
            selnm = constp.tile([NB, NRT * P], BF16, tag="selnm")
            nc.gpsimd.iota(selnm[:], pattern=[[4, NRT], [1, 4], [0, R]], base=0,
                           channel_multiplier=0,
                           allow_small_or_imprecise_dtypes=True)
            identb = constp.tile([P, P], BF16, tag="identb")
            nc.vector.tensor_scalar(identb[:], colp[:], pvec[:, 0:1], None,
                                    ALU.is_equal)
            self_sel = constp.tile([NB, NRT * P], BF16, tag="selfsel")
            nc.vector.tensor_scalar(self_sel[:], selnm[:], pvec[:NB, 0:1], None,
                                    ALU.is_equal)

            # ---- resident loads: strict need-order FIFO on the two HWDGE
            # rings so critical-path tensors never starve behind bulk.
            # (The scalar ring's first trigger is delayed ~1.3us by the Act
            # table load, so the most critical chain goes on sync.) ----
            blobb = constp.tile([P, BB_W], BF16, tag="blobb")
            nc.sync.dma_start(blobb[:], blobb_d[:])
            a2 = constp.tile([P, NRT, D], FP8, tag="a2")
            nc.sync.dma_start(a2[:], a2_d[:])
            blobf = constp.tile([P, BF_W], F32, tag="blobf")
            nc.sync.dma_start(blobf[:], blobf_d[:])
            # scalar ring: a1, then xtb, odd wu chunks, wd
            a1 = constp.tile([P, DC, NB * R], FP8, tag="a1")
            nc.scalar.dma_start(a1[:], a1_d[:])
            xtb = constp.tile([P, DC, T], BF16, tag="xtb")
            nc.scalar.dma_start(xtb[:], xtb_d[:])

            bu = blobf[:, BF_BU:BF_BU + FT]
            bd = blobf[:, BF_BD:BF_BD + DT]
            qm = blobb[:, BB_QM:BB_QM + P]
            rec = blobb[:NN, BB_REC:BB_REC + NB]

            # ---- routing: weighted one-hot scatter S[t, neuron], batched
            # over all 4 token tiles (all-bf16 for 2x DVE); the K-reduction
            # is split DVE/GpSimd ----
            iota_b = iot[:].rearrange(
                "p (o n k) -> p o n k", o=1, k=K).to_broadcast((P, TT, NN, K))
            idxw = blobb[:, BB_IDX:BB_IDX + 2 * K * TT].rearrange(
                "p (t k) -> p t k", t=TT)
            idx_b = idxw[:, :, 0:K].rearrange(
                "p t (o k) -> p t o k", o=1).to_broadcast((P, TT, NN, K))
            w_b = idxw[:, :, K:2 * K].rearrange(
                "p t (o k) -> p t o k", o=1).to_broadcast((P, TT, NN, K))
            sk = small.tile([P, TT, NN, K], BF16, tag="sk")
            s_red = small.tile([P, TT, NN], BF16, tag="sred")
            HT = TT // 2
            with nc.allow_low_precision("s values are sums of <=8 weights"):
                nc.vector.tensor_tensor(sk[:], iota_b, idx_b, ALU.is_equal)
                nc.vector.tensor_tensor(sk[:], sk[:], w_b, ALU.mult)
                nc.vector.reduce_sum(s_red[:, :HT, :], sk[:, :HT, :, :], axis=AX)
                # GpSimd takes the other half of the K-reduction via
                # pairwise adds (it has no free-axis reduce or is_equal)
                nc.gpsimd.tensor_tensor(sk[:, HT:, :, 0:4], sk[:, HT:, :, 0:4],
                                        sk[:, HT:, :, 4:8], ALU.add)
                nc.gpsimd.tensor_tensor(sk[:, HT:, :, 0:2], sk[:, HT:, :, 0:2],
                                        sk[:, HT:, :, 2:4], ALU.add)
                nc.gpsimd.tensor_tensor(s_red[:, HT:, :], sk[:, HT:, :, 0],
                                        sk[:, HT:, :, 1], ALU.add)


            # ---- softmax over the 16-basis axis of the recipe table ----
            esb = small.tile([NN, NB], F32, tag="esb")
            ssum = small.tile([NN, 1], F32, tag="ssum")
            nc.scalar.activation(esb[:], rec, AF.Exp, accum_out=ssum[:])
            rsum = small.tile([NN, 1], F32, tag="rsum")
            nc.vector.reciprocal(rsum[:], ssum[:])
            rsum2 = small.tile([NN, 1], F32, tag="rsum2")
            nc.vector.tensor_scalar_mul(rsum2[:], rsum[:], SIGR)
            recs = constp.tile([NN, NB], BF16, tag="recs")
            nc.scalar.activation(recs[:], esb[:], AF.Copy, scale=rsum2[:, 0:1])
            # RSEL[n', i*128+m] = SIGR * recipe_soft[n', 4i + m//32]: the
            # recipe columns replicated over r -- computed off the critical
            # path, replacing the rt/recipeT hop entirely
            rt_ps = psums.tile([NB, NN], BF16, tag="rhps", name="recsT")
            nc.tensor.transpose(rt_ps[:], recs[:], identb[:NN, :NN])
            recsT = constp.tile([NB, NN], BF16, tag="recsT")
            nc.vector.tensor_copy(recsT[:], rt_ps[:])
            rsel_ps = psums.tile([NN, NRT * P], F32, tag="rtps", name="rsel")
            nc.tensor.matmul(rsel_ps[:], recsT[:], self_sel[:],
                             start=True, stop=True)
            rsel = constp.tile([NN, NRT * P], BF16, tag="rsel")
            nc.scalar.activation(rsel[:], rsel_ps[:], AF.Copy)

            # anchor read keeps the warm-up matmuls from being dead-code
            # eliminated (Act engine; DVE is the busy one here)
            warm_anchor = tmpp.tile([P, 1], F32, tag="fanchor", name="wanchor")
            nc.scalar.activation(warm_anchor[:], warm_ps[:, 0:1], AF.Copy)

            # scatter transposes, then RepR[(n,r), t] = RSEL^T @ st.
            # rp psums ping-pong between two slots; the PSUM->SBUF copies
            # split between Act and DVE.
            st_sb = constp.tile([NN, T], BF16, tag="st")
            for tt in range(TT):
                # alternate between two serial psum slots for ping-pong
                stp = psums.tile([NN, P], BF16, tag="rhps" if tt % 2 else "rtps",
                                 name=f"stp{tt}")
                nc.tensor.transpose(stp[:], s_red[:, tt, :], identb)
                if tt % 2 == 0:
                    nc.scalar.activation(st_sb[:, ts(tt, P)], stp[:], AF.Copy)
                else:
                    nc.vector.tensor_copy(st_sb[:, ts(tt, P)], stp[:])
            repr_sb = []
            for i in range(NRT):
                rp = psums.tile([P, T], F32, tag="htps" if i % 2 else "rtps",
                                name=f"rp{i}")
                nc.tensor.matmul(rp[:], rsel[:, ts(i, P)], st_sb[:],
                                 start=True, stop=True)
                rr = constp.tile([P, T], BF16, tag=f"repr{i}", name=f"repr{i}")
                if i % 2 == 0:
                    nc.scalar.activation(rr[:], rp[:], AF.Copy)
                else:
                    nc.vector.tensor_copy(rr[:], rp[:])
                repr_sb.append(rr)

            filler("t", 6)

            # ---- YT = A1^T @ xT, fp8 DoubleRow (2 d-chunks per matmul) ----
            yt_ps = [psum.tile([P, T], F32, tag="ps", name=f"yt{i}")
                     for i in range(NRT)]
            for i in range(NRT):
                for cp in range(DC // 2):
                    nc.tensor.matmul(yt_ps[i][:],
                                     a1[:, 2 * cp:2 * cp + 2, ts(i, P)],
                                     x8[:, 2 * cp:2 * cp + 2, :],
                                     start=(cp == 0), stop=(cp == DC // 2 - 1),
                                     perf_mode=DR)
            filler("a", 8)

            # ---- WYT = YT * RepR;  rh = 32768*h via one accumulated matmul
            # (M folds the n-sum AND the r-replication: M[p',p] =
            #  32768*[p'%32 == p%32]) ----
            rh_ps = psums.tile([P, T], F32, tag="rhps")
            wyt = [mid.tile([P, T], BF16, tag=f"mid{i}", name=f"wyt{i}")
                   for i in range(NRT)]
            for i in range(NRT):
                nc.vector.tensor_mul(out=wyt[i][:], in0=yt_ps[i][:],
                                     in1=repr_sb[i][:])
                nc.tensor.matmul(rh_ps[:], qm, wyt[i][:],
                                 start=(i == 0), stop=(i == NRT - 1))
            filler("c", 12)

            # ---- CT = RepH * RepR (fp8) ----
            ct8 = constp.tile([P, NRT, T], FP8, tag="ct8")
            with nc.allow_low_precision("ct is alpha-damped, fp8 is enough"):
                for i in range(NRT):
                    nc.vector.tensor_mul(out=ct8[:, i, :], in0=rh_ps[:],
                                         in1=repr_sb[i][:])
            # xf = 8192*(x + alpha*delta): psum copies split Act/DVE,
            # bf16 adds on DVE (2x rate), in place over xtb
            for half in range(2):
                dts = range(half * 4, half * 4 + 4)
                dl_ps = {dt: psum.tile([P, T], F32, tag="ps", name=f"dl{dt}")
                         for dt in dts}
                # j outer so the first delta matmuls only need ct8[0:2]
                for j in range(NRT // 2):
                    for dt in dts:
                        nc.tensor.matmul(dl_ps[dt][:],
                                         a2[:, 2 * j:2 * j + 2, ts(dt, P)],
                                         ct8[:, 2 * j:2 * j + 2, :],
                                         start=(j == 0), stop=(j == NRT // 2 - 1),
                                         perf_mode=DR)
                for dt in dts:
                    dl_sb = tmpp.tile([P, T], BF16, tag="dl", name=f"dl_sb{dt}")
                    if dt % 4 != 3:
                        nc.scalar.activation(dl_sb[:], dl_ps[dt][:], AF.Copy)
                    else:
                        nc.vector.tensor_copy(dl_sb[:], dl_ps[dt][:])
                    nc.vector.tensor_add(out=xtb[:, dt, :], in0=dl_sb[:],
                                         in1=xtb[:, dt, :])
            filler("e", 40)

            # ---- FFN up + exact gelu (descale 2^-13 folded into Act) ----
            g = constp.tile([P, FT, T], BF16, tag="g")
            for ftp in range(FT // 2):
                wu = stream.tile([P, 2, DC, P], BF16, tag="wu", name=f"wu{ftp}")
                # alternate rings so the wu stream arrives 2x faster
                (nc.sync if ftp % 2 == 0 else nc.scalar).dma_start(
                    wu[:], wu_d[ftp])
                for j in range(2):
                    ft = 2 * ftp + j
                    u_ps = psum.tile([P, T], F32, tag="ps", name=f"u{ft}")
                    for dc in range(DC):
                        nc.tensor.matmul(u_ps[:], wu[:, j, dc, :], xtb[:, dc, :],
                                         start=(dc == 0), stop=(dc == DC - 1))
                    nc.scalar.activation(g[:, ft, :], u_ps[:], AF.Gelu,
                                         bias=bu[:, ft:ft + 1], scale=GS)

            # ---- FFN down + bias (bias via Act Copy; the last d-tile is
            # computed in two half-token groups so its bias/store overlaps
            # the remaining matmuls, shortening the kernel tail) ----
            for dt in range(DT):
                o_ps = psum.tile([P, T], F32, tag="ps", name=f"o{dt}")
                wds = []
                for h in range(2):
                    wd = wdstream.tile([P, FT // 2, P], BF16, tag="wd",
                                       name=f"wd{dt}_{h}")
                    nc.scalar.dma_start(wd[:], wd_d[dt * 2 + h])
                    wds.append(wd)
                halves = [(0, T)] if dt < DT - 1 else [(0, T // 2), (T // 2, T)]
                for c0, c1 in halves:
                    ops = o_ps if c0 == 0 else psums.tile([P, T], F32,
                                                          tag="htps", name="o7b")
                    for h in range(2):
                        for fc in range(FT // 2):
                            fcg = h * (FT // 2) + fc
                            nc.tensor.matmul(ops[:, c0:c1], wds[h][:, fc, :],
                                             g[:, fcg, c0:c1],
                                             start=(fcg == 0), stop=(fcg == FT - 1))
                    ot = otp.tile([P, c1 - c0], F32, tag="ot" if c1 - c0 == T
                                  else f"oth{c0}", name=f"ot{dt}_{c0}")
                    nc.vector.tensor_scalar_add(ot[:], ops[:, c0:c1],
                                                bd[:, dt:dt + 1])
                    nc.sync.dma_start(out_d[:, dt, c0:c1], ot[:])

    nc.finalize()
    return nc


def _get_nc():
    if _BUILT[0] is None:
        _BUILT[0] = _build_nc()
    return _BUILT[0]


def kernel(x, neuron_idx, neuron_weights, neuron_recipe, basis_A,
           w_up_w, w_up_b, w_down_w, w_down_b, alpha):
    import ml_dtypes
    nc = _get_nc()
    bf16 = ml_dtypes.bfloat16
    fp8 = ml_dtypes.float8_e4m3  # trn2 fp8e4: IEEE-style, max normal 240

    def to8(a):
        return np.clip(a, -F8MAX, F8MAX).astype(fp8)

    x = np.asarray(x, dtype=np.float32).reshape(NCORES * T, D)
    idxf = np.asarray(neuron_idx).astype(np.float32).reshape(NCORES * T, K)
    wgt = np.asarray(neuron_weights, dtype=np.float32).reshape(NCORES * T, K)
    rec = np.asarray(neuron_recipe, dtype=np.float32)
    bA = np.asarray(basis_A, dtype=np.float32)
    wu = np.asarray(w_up_w, dtype=np.float32)
    bu_in = np.asarray(w_up_b, dtype=np.float32)
    wd = np.asarray(w_down_w, dtype=np.float32)
    bd_in = np.asarray(w_down_b, dtype=np.float32)
    alpha_f = float(np.asarray(alpha, dtype=np.float32))

    # replicated operands, packed into the on-device layouts
    a1 = to8(np.ascontiguousarray(
        bA.transpose(1, 0, 2).reshape(D, NB * R)
        .reshape(DC, P, NB * R).transpose(1, 0, 2)) * SA)
    a2 = to8(np.ascontiguousarray(
        bA.transpose(0, 2, 1).reshape(NB * R, D)
        .reshape(NRT, P, D).transpose(1, 0, 2)) * (S2 * alpha_f))
    wu_p = np.ascontiguousarray(
        wu.reshape(DC, P, FT // 2, 2, P).transpose(2, 1, 3, 0, 4)
    ).astype(bf16)
    wd_p = np.ascontiguousarray(
        wd.reshape(2, FT // 2, P, DT, P).transpose(3, 0, 2, 1, 4)
        .reshape(DT * 2, P, FT // 2, P)).astype(bf16)

    blobf = np.zeros((P, BF_W), dtype=np.float32)
    blobf[:, BF_BU:BF_BU + FT] = bu_in.reshape(FT, P).T
    blobf[:, BF_BD:BF_BD + DT] = bd_in.reshape(DT, P).T

    blobb_base = np.zeros((P, BB_W), dtype=np.float32)
    blobb_base[:, BB_QM:BB_QM + P] = SIGM * (
        np.arange(P)[:, None] % R == np.arange(P)[None, :] % R)
    blobb_base[:NN, BB_REC:BB_REC + NB] = rec

    shared = {
        "blobf": blobf,
        "a1": a1, "a2": a2, "wu": wu_p, "wd": wd_p,
    }
    in_maps = []
    idxw = np.concatenate([idxf, wgt], axis=1)  # [N*T, 16]
    for c in range(NCORES):
        xc = x[c * T:(c + 1) * T]  # [T, D]
        xtc = np.ascontiguousarray(xc.T.reshape(DC, P, T).transpose(1, 0, 2))
        blobb = blobb_base.copy()
        blobb[:, BB_IDX:BB_IDX + 2 * K * TT] = (
            idxw[c * T:(c + 1) * T].reshape(TT, P, 2 * K).transpose(1, 0, 2)
            .reshape(P, 2 * K * TT))
        in_maps.append({"x8": to8(xtc * SX), "xtb": (xtc * XS).astype(bf16),
                        "blobb": blobb.astype(bf16), **shared})

    res = run_bass_kernel_spmd(nc, in_maps, core_ids=list(range(NCORES)))

    out = np.empty((NCORES * T, D), dtype=np.float32)
    for c in range(NCORES):
        ot = res.results[c]["outT"]  # [P, DT, T]
        out[c * T:(c + 1) * T] = ot.transpose(1, 0, 2).reshape(D, T).T
    return out.reshape(2, 2048, D)
